# revision 7
# baseline (speedup 1.0000x reference)
"""GAT edge-softmax kernel for 8 trn2 NeuronCores.

Strategy (per sharding hint): edges bucketed by destination-row range
(12500 rows/core) so segment softmax is core-local. Within a core, rows are
sorted by degree and packed into 128-lane groups padded to the group max
degree (rounded to 8) -> dense [128, W] "row-stripe" layout where every
per-edge op is affine.

Launch A: row-sharded matvec s = x @ att halves on PE (the memory-roofline
term: each core reads its 12.5MB x shard once).
Launch B: alpha = leaky_relu(s_src[row] + s_dst[col]) -> exp -> per-row
segment sums (free-dim reduces batched by stripe-length class) -> normalize.
s_src[row] and 1/denom broadcasts are zero-stride affine copies; pad slots
carry -1e30 so exp() kills them. The softmax max-subtraction cancels
algebraically and alpha is bounded (|s| <= ~4), so it is omitted.

Host does the sharding/unsharding: bucketing, degree sort, slot assignment,
the s_dst value resharding between launches, and the final unpermute.
"""

import numpy as np

import concourse.bass as bass
import concourse.bacc as bacc
import concourse.mybir as mybir
from concourse.tile import TileContext
from concourse.bass_utils import run_bass_kernel_spmd

N_NODES = 100000
N_EDGES = 3200000
C = 256
NEG_SLOPE = 0.2
NCORES = 8
RPC = N_NODES // NCORES          # rows per core
P = 128
NGRP = (RPC + P - 1) // P        # 98 row groups per core
RPAD = NGRP * P                  # 12544
NEG_BIG = np.float32(-1e30)

EXEC_NS = {"A": None, "B": None}


# matvec split: ranks [0, NPE) on PE (d-major layout, padded to NPEP cols),
# ranks [NPE, RPC) on DVE (node-major layout, NDVE = 128*NPP nodes)
NPE = 7508
NPEP = 7680
NPP = 39
NDVE = P * NPP
assert NPE + NDVE == RPC


def _build_launch_a():
    nc = bacc.Bacc("TRN2", target_bir_lowering=False)
    f32 = mybir.dt.float32
    att_d = nc.dram_tensor("att4", [P, 4], f32, kind="ExternalInput")
    attr_d = nc.dram_tensor("attr", [P, 2 * C], f32, kind="ExternalInput")
    xh0_d = nc.dram_tensor("xh0", [P, NPEP], f32, kind="ExternalInput")
    xh1_d = nc.dram_tensor("xh1", [P, NPEP], f32, kind="ExternalInput")
    xn_d = nc.dram_tensor("xn", [P, NPP * C], f32, kind="ExternalInput")
    s_d = nc.dram_tensor("s", [2, NPEP], f32, kind="ExternalOutput")
    sdve_d = nc.dram_tensor("sdve", [P, 2 * NPP], f32, kind="ExternalOutput")
    CH = 512
    NCH = NPEP // CH
    DCH = 13  # nodes per DVE chunk (3 chunks of 13)
    with TileContext(nc) as tc:
        with (
            tc.tile_pool(name="cst", bufs=1) as cst,
            tc.tile_pool(name="xs", bufs=4) as xs,
            tc.tile_pool(name="xnp", bufs=2) as xnp,
            tc.tile_pool(name="tmp", bufs=2) as tmpp,
            tc.tile_pool(name="acc", bufs=1) as acc,
            tc.tile_pool(name="ps", bufs=4, space="PSUM") as ps,
        ):
            att_t = cst.tile([P, 4], f32)
            attr_t = cst.tile([P, 2 * C], f32)
            nc.sync.dma_start(att_t[:], att_d[:])
            nc.sync.dma_start(attr_t[:], attr_d[:])
            s_sb = acc.tile([2, NPEP], f32)
            sdve = acc.tile([P, 2 * NPP], f32)
            for ch in range(NCH):
                sl = slice(ch * CH, (ch + 1) * CH)
                x0 = xs.tile([P, CH], f32, tag="x0")
                x1 = xs.tile([P, CH], f32, tag="x1")
                nc.sync.dma_start(x0[:], xh0_d[:, sl])
                nc.sync.dma_start(x1[:], xh1_d[:, sl])
                pt = ps.tile([2, CH], f32)
                nc.tensor.matmul(pt[:], att_t[:, 0:2], x0[:], start=True, stop=False)
                nc.tensor.matmul(pt[:], att_t[:, 2:4], x1[:], start=False, stop=True)
                nc.vector.tensor_copy(s_sb[:, sl], pt[:])
            nc.sync.dma_start(s_d[:], s_sb[:])
            # DVE path: s[node] = sum_d xn[p, i, d] * att[v*C + d]
            for dc in range(NPP // DCH):
                nsl = slice(dc * DCH * C, (dc + 1) * DCH * C)
                xt = xnp.tile([P, DCH * C], f32, tag="xn")
                nc.sync.dma_start(xt[:], xn_d[:, nsl])
                for v in range(2):
                    tmp = tmpp.tile([P, DCH * C], f32, tag="tm")
                    a = attr_t[:, v * C : (v + 1) * C]
                    a_ap = bass.AP(a.tensor, a.offset, [a.ap[0], [0, DCH], [1, C]])
                    x_ap = bass.AP(
                        xt[:].tensor, xt[:].offset, [xt[:].ap[0], [C, DCH], [1, C]]
                    )
                    t_ap = bass.AP(
                        tmp[:].tensor, tmp[:].offset, [tmp[:].ap[0], [C, DCH], [1, C]]
                    )
                    nc.vector.tensor_tensor(t_ap, x_ap, a_ap, op=mybir.AluOpType.mult)
                    o = sdve[:, 2 * dc * DCH + v :]
                    o_ap = bass.AP(o.tensor, o.offset, [o.ap[0], [2, DCH]])
                    nc.vector.reduce_sum(o_ap, t_ap, axis=mybir.AxisListType.X)
            nc.sync.dma_start(sdve_d[:], sdve[:])
    nc.compile()
    return nc


def _build_launch_b(W, classes):
    """classes: list of (g0, g1, off0, L) — groups [g0,g1) share stripe len L,
    their slots occupy [off0, off0 + (g1-g0)*L)."""
    nc = bacc.Bacc("TRN2", target_bir_lowering=False)
    b_d = nc.dram_tensor("bvals", [P, W], mybir.dt.float32, kind="ExternalInput")
    ssrc_d = nc.dram_tensor("ssrc", [RPAD], mybir.dt.float32, kind="ExternalInput")
    out_d = nc.dram_tensor("out", [P, W], mybir.dt.float32, kind="ExternalOutput")
    f32 = mybir.dt.float32
    with TileContext(nc) as tc:
        with (
            tc.tile_pool(name="ec", bufs=1) as ec,
            tc.tile_pool(name="scr", bufs=4) as scr,
            tc.tile_pool(name="sm", bufs=1) as sm,
        ):
            ssrc = sm.tile([P, NGRP], f32)
            den = sm.tile([P, NGRP], f32)
            inv = sm.tile([P, NGRP], f32)
            # ssrc_d is rank-major: entry (g*128 + p) -> ssrc[p, g]
            nc.sync.dma_start(ssrc[:], ssrc_d[:].rearrange("(g p) -> p g", p=P))

            def bcast_ap(src_tile, g0, g1, L):
                s = src_tile[:, g0:g1]
                return bass.AP(s.tensor, s.offset, [s.ap[0], s.ap[1], [0, L]])

            def grp_ap(t, ng, L):
                a = t[:, : ng * L]
                return bass.AP(a.tensor, a.offset, [a.ap[0], [L, ng], [1, L]])

            etiles = []
            for ci, (g0, g1, off0, L) in enumerate(classes):
                ng = g1 - g0
                n = ng * L
                t = ec.tile([P, n], f32, tag=f"e{ci}")
                u = scr.tile([P, n], f32, tag="u")
                nc.sync.dma_start(t[:], b_d[:, off0 : off0 + n])
                # u = s_src broadcast over stripes (on ACT engine)
                nc.scalar.copy(grp_ap(u, ng, L), bcast_ap(ssrc, g0, g1, L))
                nc.vector.tensor_tensor(t[:], t[:], u[:], op=mybir.AluOpType.add)
                # leaky_relu: max(z, 0.2*z) (exact for slope<1)
                nc.scalar.mul(u[:], t[:], NEG_SLOPE)
                nc.vector.tensor_tensor(t[:], t[:], u[:], op=mybir.AluOpType.max)
                nc.scalar.activation(t[:], t[:], mybir.ActivationFunctionType.Exp)
                nc.vector.reduce_sum(
                    den[:, g0:g1], grp_ap(t, ng, L), axis=mybir.AxisListType.X
                )
                etiles.append(t)
            # zero-degree rows give denom=0 -> inf/NaN only in pad slots,
            # which the host discards.
            nc.vector.reciprocal(inv[:], den[:])
            for ci, (g0, g1, off0, L) in enumerate(classes):
                ng = g1 - g0
                n = ng * L
                t = etiles[ci]
                v = scr.tile([P, n], f32, tag="v")
                nc.scalar.copy(grp_ap(v, ng, L), bcast_ap(inv, g0, g1, L))
                nc.vector.tensor_tensor(t[:], t[:], v[:], op=mybir.AluOpType.mult)
                nc.sync.dma_start(out_d[:, off0 : off0 + n], t[:])
    nc.compile()
    return nc


def kernel(x, att, edge_index):
    x = np.ascontiguousarray(np.asarray(x, dtype=np.float32))
    att = np.asarray(att, dtype=np.float32).reshape(2 * C)
    row = np.asarray(edge_index[0], dtype=np.int64)
    col = np.asarray(edge_index[1], dtype=np.int64)

    # ---- host: shard edges by destination-row bucket; degree-sort rows ----
    core_of = row // RPC
    per_core = []  # dicts with everything per core
    Lg_per_core = np.zeros((NCORES, NGRP), dtype=np.int64)
    for k in range(NCORES):
        m = np.flatnonzero(core_of == k)
        r = row[m] - k * RPC
        deg = np.bincount(r, minlength=RPC)
        rorder = np.argsort(-deg, kind="stable")      # rank -> local row
        rank_of_row = np.empty(RPC, dtype=np.int64)
        rank_of_row[rorder] = np.arange(RPC)
        degs = deg[rorder]                            # degree by rank (desc)
        gmax = degs[::P][:NGRP]                       # max degree per group
        Lg = np.maximum(8, ((gmax + 7) // 8) * 8)
        Lg_per_core[k] = Lg
        per_core.append(dict(m=m, r=r, rorder=rorder, rank_of_row=rank_of_row))

    Lg = Lg_per_core.max(axis=0)                      # shared stripe lengths
    off = np.zeros(NGRP + 1, dtype=np.int64)
    off[1:] = np.cumsum(Lg)
    W = int(off[-1])
    # classes: runs of equal L
    classes = []
    g0 = 0
    for g in range(1, NGRP + 1):
        if g == NGRP or Lg[g] != Lg[g0]:
            classes.append((int(g0), int(g), int(off[g0]), int(Lg[g0])))
            g0 = g

    # per-core slot assignment
    for k in range(NCORES):
        d = per_core[k]
        rk = d["rank_of_row"][d["r"]]
        eorder = np.argsort(rk, kind="stable")        # edges sorted by rank
        rk_s = rk[eorder]
        uniq, counts = np.unique(rk_s, return_counts=True)
        starts = np.zeros(len(uniq), dtype=np.int64)
        starts[1:] = np.cumsum(counts)[:-1]
        pos = np.arange(len(rk_s)) - np.repeat(starts, counts)
        g = rk_s // P
        lane = rk_s % P
        wslot = off[g] + pos
        d.update(eorder=eorder, lane=lane, wslot=wslot)

    # ---- launch A: matvec on device ----
    nc_a = _build_launch_a()
    att4 = np.empty((P, 4), dtype=np.float32)
    att4[:, 0] = att[0:128]
    att4[:, 1] = att[256:384]
    att4[:, 2] = att[128:256]
    att4[:, 3] = att[384:512]
    attr = np.tile(att[None, :], (P, 1)).astype(np.float32)
    in_maps_a = []
    for k in range(NCORES):
        xp = x[k * RPC + per_core[k]["rorder"], :]    # rank-ordered shard
        xh = np.zeros((2, P, NPEP), dtype=np.float32)
        xh[0, :, :NPE] = xp[:NPE, :128].T
        xh[1, :, :NPE] = xp[:NPE, 128:].T
        xn = np.ascontiguousarray(
            xp[NPE:].reshape(NPP, P, C).transpose(1, 0, 2).reshape(P, NPP * C)
        )
        in_maps_a.append(
            dict(
                att4=att4,
                attr=attr,
                xh0=np.ascontiguousarray(xh[0]),
                xh1=np.ascontiguousarray(xh[1]),
                xn=xn,
            )
        )
    res_a = run_bass_kernel_spmd(
        nc_a, in_maps_a, core_ids=list(range(NCORES)), trace=True
    )
    EXEC_NS["A"] = res_a.exec_time_ns

    # assemble s_dst in original node ids; keep s_src in rank order
    s_dst_all = np.empty(N_NODES, dtype=np.float32)
    ssrc_rank = []
    for k in range(NCORES):
        s = res_a.results[k]["s"]
        sdve = res_a.results[k]["sdve"]
        sr = np.zeros(RPAD, dtype=np.float32)
        sr[:NPE] = s[0, :NPE]
        sr[NPE:RPC] = sdve[:, 0::2].T.reshape(-1)
        ssrc_rank.append(sr)
        sd = np.empty(RPC, dtype=np.float32)
        sd[:NPE] = s[1, :NPE]
        sd[NPE:] = sdve[:, 1::2].T.reshape(-1)
        s_dst_all[k * RPC + per_core[k]["rorder"]] = sd

    # ---- host reshard: expand s_dst values into the row-stripe layout ----
    nc_b = _build_launch_b(W, classes)
    in_maps_b = []
    for k in range(NCORES):
        d = per_core[k]
        b = np.full((P, W), NEG_BIG, dtype=np.float32)
        b[d["lane"], d["wslot"]] = s_dst_all[col[d["m"][d["eorder"]]]]
        in_maps_b.append(dict(bvals=b, ssrc=ssrc_rank[k]))
    res_b = run_bass_kernel_spmd(
        nc_b, in_maps_b, core_ids=list(range(NCORES)), trace=True
    )
    EXEC_NS["B"] = res_b.exec_time_ns

    # ---- host unshard: pick real slots back into original edge order ----
    out = np.empty(N_EDGES, dtype=np.float32)
    for k in range(NCORES):
        d = per_core[k]
        dev = res_b.results[k]["out"]
        out[d["m"][d["eorder"]]] = dev[d["lane"], d["wslot"]]
    return out[None, :]


# revision 8
# speedup vs baseline: 1.0362x; 1.0362x over previous
"""GAT edge-softmax kernel for 8 trn2 NeuronCores.

Strategy (per sharding hint): edges bucketed by destination-row range
(12500 rows/core) so segment softmax is core-local. Within a core, rows are
sorted by degree and packed into 128-lane groups padded to the group max
degree (rounded to 8) -> dense [128, W] "row-stripe" layout where every
per-edge op is affine.

Launch A: row-sharded matvec s = x @ att halves on PE (the memory-roofline
term: each core reads its 12.5MB x shard once).
Launch B: alpha = leaky_relu(s_src[row] + s_dst[col]) -> exp -> per-row
segment sums (free-dim reduces batched by stripe-length class) -> normalize.
s_src[row] and 1/denom broadcasts are zero-stride affine copies; pad slots
carry -1e30 so exp() kills them. The softmax max-subtraction cancels
algebraically and alpha is bounded (|s| <= ~4), so it is omitted.

Host does the sharding/unsharding: bucketing, degree sort, slot assignment,
the s_dst value resharding between launches, and the final unpermute.
"""

import numpy as np

import concourse.bass as bass
import concourse.bacc as bacc
import concourse.mybir as mybir
from concourse.tile import TileContext
from concourse.bass_utils import run_bass_kernel_spmd

N_NODES = 100000
N_EDGES = 3200000
C = 256
NEG_SLOPE = 0.2
NCORES = 8
RPC = N_NODES // NCORES          # rows per core
P = 128
NGRP = (RPC + P - 1) // P        # 98 row groups per core
RPAD = NGRP * P                  # 12544
NEG_BIG = np.float32(-1e30)

EXEC_NS = {"A": None, "B": None}


# matvec split: ranks [0, NPE) on PE (d-major layout, padded to NPEP cols),
# ranks [NPE, RPC) on DVE (node-major layout, NDVE = 128*NPP nodes)
NPE = 7508
NPEP = 7680
NPP = 39
NDVE = P * NPP
assert NPE + NDVE == RPC


def _build_launch_a():
    nc = bacc.Bacc("TRN2", target_bir_lowering=False)
    f32 = mybir.dt.float32
    att_d = nc.dram_tensor("att4", [P, 4], f32, kind="ExternalInput")
    attr_d = nc.dram_tensor("attr", [P, 2 * C], f32, kind="ExternalInput")
    xh0_d = nc.dram_tensor("xh0", [P, NPEP], f32, kind="ExternalInput")
    xh1_d = nc.dram_tensor("xh1", [P, NPEP], f32, kind="ExternalInput")
    xn_d = nc.dram_tensor("xn", [P, NPP * C], f32, kind="ExternalInput")
    s_d = nc.dram_tensor("s", [2, NPEP], f32, kind="ExternalOutput")
    sdve_d = nc.dram_tensor("sdve", [P, 2 * NPP], f32, kind="ExternalOutput")
    CH = 512
    NCH = NPEP // CH
    DCH = 13  # nodes per DVE chunk (3 chunks of 13)
    with TileContext(nc) as tc:
        with (
            tc.tile_pool(name="cst", bufs=1) as cst,
            tc.tile_pool(name="xs", bufs=4) as xs,
            tc.tile_pool(name="xnp", bufs=2) as xnp,
            tc.tile_pool(name="tmp", bufs=2) as tmpp,
            tc.tile_pool(name="acc", bufs=1) as acc,
            tc.tile_pool(name="ps", bufs=4, space="PSUM") as ps,
        ):
            att_t = cst.tile([P, 4], f32)
            attr_t = cst.tile([P, 2 * C], f32)
            nc.sync.dma_start(att_t[:], att_d[:])
            nc.sync.dma_start(attr_t[:], attr_d[:])
            s_sb = acc.tile([2, NPEP], f32)
            sdve = acc.tile([P, 2 * NPP], f32)
            for ch in range(NCH):
                sl = slice(ch * CH, (ch + 1) * CH)
                x0 = xs.tile([P, CH], f32, tag="x0")
                x1 = xs.tile([P, CH], f32, tag="x1")
                nc.sync.dma_start(x0[:], xh0_d[:, sl])
                nc.sync.dma_start(x1[:], xh1_d[:, sl])
                pt = ps.tile([2, CH], f32)
                nc.tensor.matmul(pt[:], att_t[:, 0:2], x0[:], start=True, stop=False)
                nc.tensor.matmul(pt[:], att_t[:, 2:4], x1[:], start=False, stop=True)
                nc.scalar.copy(s_sb[:, sl], pt[:])
            nc.sync.dma_start(s_d[:], s_sb[:])
            # DVE path: s[node] = sum_d xn[p, i, d] * att[v*C + d]
            for dc in range(NPP // DCH):
                nsl = slice(dc * DCH * C, (dc + 1) * DCH * C)
                xt = xnp.tile([P, DCH * C], f32, tag="xn")
                nc.sync.dma_start(xt[:], xn_d[:, nsl])
                for v in range(2):
                    tmp = tmpp.tile([P, DCH * C], f32, tag="tm")
                    a = attr_t[:, v * C : (v + 1) * C]
                    a_ap = bass.AP(a.tensor, a.offset, [a.ap[0], [0, DCH], [1, C]])
                    x_ap = bass.AP(
                        xt[:].tensor, xt[:].offset, [xt[:].ap[0], [C, DCH], [1, C]]
                    )
                    t_ap = bass.AP(
                        tmp[:].tensor, tmp[:].offset, [tmp[:].ap[0], [C, DCH], [1, C]]
                    )
                    nc.vector.tensor_tensor(t_ap, x_ap, a_ap, op=mybir.AluOpType.mult)
                    o = sdve[:, 2 * dc * DCH + v :]
                    o_ap = bass.AP(o.tensor, o.offset, [o.ap[0], [2, DCH]])
                    nc.vector.reduce_sum(o_ap, t_ap, axis=mybir.AxisListType.X)
            nc.sync.dma_start(sdve_d[:], sdve[:])
    nc.compile()
    return nc


def _build_launch_b(W, classes):
    """classes: list of (g0, g1, off0, L) — groups [g0,g1) share stripe len L,
    their slots occupy [off0, off0 + (g1-g0)*L)."""
    nc = bacc.Bacc("TRN2", target_bir_lowering=False)
    b_d = nc.dram_tensor("bvals", [P, W], mybir.dt.float32, kind="ExternalInput")
    ssrc_d = nc.dram_tensor("ssrc", [RPAD], mybir.dt.float32, kind="ExternalInput")
    out_d = nc.dram_tensor("out", [P, W], mybir.dt.float32, kind="ExternalOutput")
    f32 = mybir.dt.float32
    with TileContext(nc) as tc:
        with (
            tc.tile_pool(name="ec", bufs=1) as ec,
            tc.tile_pool(name="scr", bufs=4) as scr,
            tc.tile_pool(name="sm", bufs=1) as sm,
        ):
            ssrc = sm.tile([P, NGRP], f32)
            den = sm.tile([P, NGRP], f32)
            inv = sm.tile([P, NGRP], f32)
            # ssrc_d is rank-major: entry (g*128 + p) -> ssrc[p, g]
            nc.sync.dma_start(ssrc[:], ssrc_d[:].rearrange("(g p) -> p g", p=P))

            def bcast_ap(src_tile, g0, g1, L):
                s = src_tile[:, g0:g1]
                return bass.AP(s.tensor, s.offset, [s.ap[0], s.ap[1], [0, L]])

            def grp_ap(t, ng, L):
                a = t[:, : ng * L]
                return bass.AP(a.tensor, a.offset, [a.ap[0], [L, ng], [1, L]])

            etiles = []
            for ci, (g0, g1, off0, L) in enumerate(classes):
                ng = g1 - g0
                n = ng * L
                t = ec.tile([P, n], f32, tag=f"e{ci}")
                u = scr.tile([P, n], f32, tag="u")
                nc.sync.dma_start(t[:], b_d[:, off0 : off0 + n])
                # u = s_src broadcast over stripes (on ACT engine)
                nc.scalar.copy(grp_ap(u, ng, L), bcast_ap(ssrc, g0, g1, L))
                nc.vector.tensor_tensor(t[:], t[:], u[:], op=mybir.AluOpType.add)
                # leaky_relu: max(z, 0.2*z) (exact for slope<1)
                nc.scalar.mul(u[:], t[:], NEG_SLOPE)
                nc.vector.tensor_tensor(t[:], t[:], u[:], op=mybir.AluOpType.max)
                nc.scalar.activation(t[:], t[:], mybir.ActivationFunctionType.Exp)
                nc.vector.reduce_sum(
                    den[:, g0:g1], grp_ap(t, ng, L), axis=mybir.AxisListType.X
                )
                etiles.append(t)
            # zero-degree rows give denom=0 -> inf/NaN only in pad slots,
            # which the host discards.
            nc.vector.reciprocal(inv[:], den[:])
            for ci, (g0, g1, off0, L) in enumerate(classes):
                ng = g1 - g0
                n = ng * L
                t = etiles[ci]
                v = scr.tile([P, n], f32, tag="v")
                nc.scalar.copy(grp_ap(v, ng, L), bcast_ap(inv, g0, g1, L))
                nc.vector.tensor_tensor(t[:], t[:], v[:], op=mybir.AluOpType.mult)
                nc.sync.dma_start(out_d[:, off0 : off0 + n], t[:])
    nc.compile()
    return nc


def kernel(x, att, edge_index):
    x = np.ascontiguousarray(np.asarray(x, dtype=np.float32))
    att = np.asarray(att, dtype=np.float32).reshape(2 * C)
    row = np.asarray(edge_index[0], dtype=np.int64)
    col = np.asarray(edge_index[1], dtype=np.int64)

    # ---- host: shard edges by destination-row bucket; degree-sort rows ----
    core_of = row // RPC
    per_core = []  # dicts with everything per core
    Lg_per_core = np.zeros((NCORES, NGRP), dtype=np.int64)
    for k in range(NCORES):
        m = np.flatnonzero(core_of == k)
        r = row[m] - k * RPC
        deg = np.bincount(r, minlength=RPC)
        rorder = np.argsort(-deg, kind="stable")      # rank -> local row
        rank_of_row = np.empty(RPC, dtype=np.int64)
        rank_of_row[rorder] = np.arange(RPC)
        degs = deg[rorder]                            # degree by rank (desc)
        gmax = degs[::P][:NGRP]                       # max degree per group
        Lg = np.maximum(8, ((gmax + 7) // 8) * 8)
        Lg_per_core[k] = Lg
        per_core.append(dict(m=m, r=r, rorder=rorder, rank_of_row=rank_of_row))

    Lg = Lg_per_core.max(axis=0)                      # shared stripe lengths
    off = np.zeros(NGRP + 1, dtype=np.int64)
    off[1:] = np.cumsum(Lg)
    W = int(off[-1])
    # classes: runs of equal L
    classes = []
    g0 = 0
    for g in range(1, NGRP + 1):
        if g == NGRP or Lg[g] != Lg[g0]:
            classes.append((int(g0), int(g), int(off[g0]), int(Lg[g0])))
            g0 = g

    # per-core slot assignment
    for k in range(NCORES):
        d = per_core[k]
        rk = d["rank_of_row"][d["r"]]
        eorder = np.argsort(rk, kind="stable")        # edges sorted by rank
        rk_s = rk[eorder]
        uniq, counts = np.unique(rk_s, return_counts=True)
        starts = np.zeros(len(uniq), dtype=np.int64)
        starts[1:] = np.cumsum(counts)[:-1]
        pos = np.arange(len(rk_s)) - np.repeat(starts, counts)
        g = rk_s // P
        lane = rk_s % P
        wslot = off[g] + pos
        d.update(eorder=eorder, lane=lane, wslot=wslot)

    # ---- launch A: matvec on device ----
    nc_a = _build_launch_a()
    att4 = np.empty((P, 4), dtype=np.float32)
    att4[:, 0] = att[0:128]
    att4[:, 1] = att[256:384]
    att4[:, 2] = att[128:256]
    att4[:, 3] = att[384:512]
    attr = np.tile(att[None, :], (P, 1)).astype(np.float32)
    in_maps_a = []
    for k in range(NCORES):
        xp = x[k * RPC + per_core[k]["rorder"], :]    # rank-ordered shard
        xh = np.zeros((2, P, NPEP), dtype=np.float32)
        xh[0, :, :NPE] = xp[:NPE, :128].T
        xh[1, :, :NPE] = xp[:NPE, 128:].T
        xn = np.ascontiguousarray(
            xp[NPE:].reshape(NPP, P, C).transpose(1, 0, 2).reshape(P, NPP * C)
        )
        in_maps_a.append(
            dict(
                att4=att4,
                attr=attr,
                xh0=np.ascontiguousarray(xh[0]),
                xh1=np.ascontiguousarray(xh[1]),
                xn=xn,
            )
        )
    res_a = run_bass_kernel_spmd(
        nc_a, in_maps_a, core_ids=list(range(NCORES)), trace=True
    )
    EXEC_NS["A"] = res_a.exec_time_ns

    # assemble s_dst in original node ids; keep s_src in rank order
    s_dst_all = np.empty(N_NODES, dtype=np.float32)
    ssrc_rank = []
    for k in range(NCORES):
        s = res_a.results[k]["s"]
        sdve = res_a.results[k]["sdve"]
        sr = np.zeros(RPAD, dtype=np.float32)
        sr[:NPE] = s[0, :NPE]
        sr[NPE:RPC] = sdve[:, 0::2].T.reshape(-1)
        ssrc_rank.append(sr)
        sd = np.empty(RPC, dtype=np.float32)
        sd[:NPE] = s[1, :NPE]
        sd[NPE:] = sdve[:, 1::2].T.reshape(-1)
        s_dst_all[k * RPC + per_core[k]["rorder"]] = sd

    # ---- host reshard: expand s_dst values into the row-stripe layout ----
    nc_b = _build_launch_b(W, classes)
    in_maps_b = []
    for k in range(NCORES):
        d = per_core[k]
        b = np.full((P, W), NEG_BIG, dtype=np.float32)
        b[d["lane"], d["wslot"]] = s_dst_all[col[d["m"][d["eorder"]]]]
        in_maps_b.append(dict(bvals=b, ssrc=ssrc_rank[k]))
    res_b = run_bass_kernel_spmd(
        nc_b, in_maps_b, core_ids=list(range(NCORES)), trace=True
    )
    EXEC_NS["B"] = res_b.exec_time_ns

    # ---- host unshard: pick real slots back into original edge order ----
    out = np.empty(N_EDGES, dtype=np.float32)
    for k in range(NCORES):
        d = per_core[k]
        dev = res_b.results[k]["out"]
        out[d["m"][d["eorder"]]] = dev[d["lane"], d["wslot"]]
    return out[None, :]


# revision 9
# speedup vs baseline: 1.1743x; 1.1332x over previous
"""GAT edge-softmax kernel for 8 trn2 NeuronCores.

Strategy (per sharding hint): edges bucketed by destination-row range
(12500 rows/core) so segment softmax is core-local. Within a core, rows are
sorted by degree and packed into 128-lane groups padded to the group max
degree (rounded to 8) -> dense [128, W] "row-stripe" layout where every
per-edge op is affine.

Launch A: row-sharded matvec s = x @ att halves on PE (the memory-roofline
term: each core reads its 12.5MB x shard once).
Launch B: alpha = leaky_relu(s_src[row] + s_dst[col]) -> exp -> per-row
segment sums (free-dim reduces batched by stripe-length class) -> normalize.
s_src[row] and 1/denom broadcasts are zero-stride affine copies; pad slots
carry -1e30 so exp() kills them. The softmax max-subtraction cancels
algebraically and alpha is bounded (|s| <= ~4), so it is omitted.

Host does the sharding/unsharding: bucketing, degree sort, slot assignment,
the s_dst value resharding between launches, and the final unpermute.
"""

import numpy as np

import concourse.bass as bass
import concourse.bacc as bacc
import concourse.mybir as mybir
from concourse.tile import TileContext
from concourse.bass_utils import run_bass_kernel_spmd

N_NODES = 100000
N_EDGES = 3200000
C = 256
NEG_SLOPE = 0.2
NCORES = 8
RPC = N_NODES // NCORES          # rows per core
P = 128
NGRP = (RPC + P - 1) // P        # 98 row groups per core
RPAD = NGRP * P                  # 12544
NEG_BIG = np.float32(-1e30)

EXEC_NS = {"A": None, "B": None}


def _build_launch_a():
    nc = bacc.Bacc("TRN2", target_bir_lowering=False)
    f32 = mybir.dt.float32
    att_d = nc.dram_tensor("att4", [P, 4], f32, kind="ExternalInput")
    xh0_d = nc.dram_tensor("xh0", [P, RPC], f32, kind="ExternalInput")
    xh1_d = nc.dram_tensor("xh1", [P, RPC], f32, kind="ExternalInput")
    s_d = nc.dram_tensor("s", [2, RPC], f32, kind="ExternalOutput")
    CH = 500
    NCH = RPC // CH
    with TileContext(nc) as tc:
        with (
            tc.tile_pool(name="cst", bufs=1) as cst,
            tc.tile_pool(name="xs", bufs=4) as xs,
            tc.tile_pool(name="acc", bufs=1) as acc,
            tc.tile_pool(name="ps", bufs=4, space="PSUM") as ps,
        ):
            att_t = cst.tile([P, 4], f32)
            nc.sync.dma_start(att_t[:], att_d[:])
            s_sb = acc.tile([2, RPC], f32)
            for ch in range(NCH):
                sl = slice(ch * CH, (ch + 1) * CH)
                x0 = xs.tile([P, CH], f32, tag="x0")
                x1 = xs.tile([P, CH], f32, tag="x1")
                nc.sync.dma_start(x0[:], xh0_d[:, sl])
                nc.sync.dma_start(x1[:], xh1_d[:, sl])
                pt = ps.tile([2, CH], f32)
                nc.tensor.matmul(pt[:], att_t[:, 0:2], x0[:], start=True, stop=False)
                nc.tensor.matmul(pt[:], att_t[:, 2:4], x1[:], start=False, stop=True)
                nc.scalar.copy(s_sb[:, sl], pt[:])
            nc.sync.dma_start(s_d[:], s_sb[:])
    nc.compile()
    return nc


def _build_launch_b(W, classes):
    """classes: list of (g0, g1, off0, L) — groups [g0,g1) share stripe len L,
    their slots occupy [off0, off0 + (g1-g0)*L)."""
    nc = bacc.Bacc("TRN2", target_bir_lowering=False)
    b_d = nc.dram_tensor("bvals", [P, W], mybir.dt.float32, kind="ExternalInput")
    ssrc_d = nc.dram_tensor("ssrc", [RPAD], mybir.dt.float32, kind="ExternalInput")
    out_d = nc.dram_tensor("out", [P, W], mybir.dt.float32, kind="ExternalOutput")
    f32 = mybir.dt.float32
    with TileContext(nc) as tc:
        with (
            tc.tile_pool(name="ec", bufs=1) as ec,
            tc.tile_pool(name="scr", bufs=4) as scr,
            tc.tile_pool(name="sm", bufs=1) as sm,
        ):
            ssrc = sm.tile([P, NGRP], f32)
            den = sm.tile([P, NGRP], f32)
            inv = sm.tile([P, NGRP], f32)
            # ssrc_d is rank-major: entry (g*128 + p) -> ssrc[p, g]
            nc.sync.dma_start(ssrc[:], ssrc_d[:].rearrange("(g p) -> p g", p=P))

            def bcast_ap(src_tile, g0, g1, L):
                s = src_tile[:, g0:g1]
                return bass.AP(s.tensor, s.offset, [s.ap[0], s.ap[1], [0, L]])

            def grp_ap(t, ng, L):
                a = t[:, : ng * L]
                return bass.AP(a.tensor, a.offset, [a.ap[0], [L, ng], [1, L]])

            etiles = []
            for ci, (g0, g1, off0, L) in enumerate(classes):
                ng = g1 - g0
                n = ng * L
                t = ec.tile([P, n], f32, tag=f"e{ci}")
                u = scr.tile([P, n], f32, tag="u")
                nc.sync.dma_start(t[:], b_d[:, off0 : off0 + n])
                # u = s_src broadcast over stripes (on ACT engine)
                nc.scalar.copy(grp_ap(u, ng, L), bcast_ap(ssrc, g0, g1, L))
                nc.vector.tensor_tensor(t[:], t[:], u[:], op=mybir.AluOpType.add)
                # leaky_relu: max(z, 0.2*z) (exact for slope<1)
                nc.scalar.mul(u[:], t[:], NEG_SLOPE)
                nc.vector.tensor_tensor(t[:], t[:], u[:], op=mybir.AluOpType.max)
                nc.scalar.activation(t[:], t[:], mybir.ActivationFunctionType.Exp)
                nc.vector.reduce_sum(
                    den[:, g0:g1], grp_ap(t, ng, L), axis=mybir.AxisListType.X
                )
                etiles.append(t)
            # zero-degree rows give denom=0 -> inf/NaN only in pad slots,
            # which the host discards.
            nc.vector.reciprocal(inv[:], den[:])
            for ci, (g0, g1, off0, L) in enumerate(classes):
                ng = g1 - g0
                n = ng * L
                t = etiles[ci]
                v = scr.tile([P, n], f32, tag="v")
                nc.scalar.copy(grp_ap(v, ng, L), bcast_ap(inv, g0, g1, L))
                nc.vector.tensor_tensor(t[:], t[:], v[:], op=mybir.AluOpType.mult)
                nc.sync.dma_start(out_d[:, off0 : off0 + n], t[:])
    nc.compile()
    return nc


def kernel(x, att, edge_index):
    x = np.ascontiguousarray(np.asarray(x, dtype=np.float32))
    att = np.asarray(att, dtype=np.float32).reshape(2 * C)
    row = np.asarray(edge_index[0], dtype=np.int64)
    col = np.asarray(edge_index[1], dtype=np.int64)

    # ---- host: shard edges by destination-row bucket; degree-sort rows ----
    core_of = row // RPC
    per_core = []  # dicts with everything per core
    Lg_per_core = np.zeros((NCORES, NGRP), dtype=np.int64)
    for k in range(NCORES):
        m = np.flatnonzero(core_of == k)
        r = row[m] - k * RPC
        deg = np.bincount(r, minlength=RPC)
        rorder = np.argsort(-deg, kind="stable")      # rank -> local row
        rank_of_row = np.empty(RPC, dtype=np.int64)
        rank_of_row[rorder] = np.arange(RPC)
        degs = deg[rorder]                            # degree by rank (desc)
        gmax = degs[::P][:NGRP]                       # max degree per group
        Lg = np.maximum(8, ((gmax + 7) // 8) * 8)
        Lg_per_core[k] = Lg
        per_core.append(dict(m=m, r=r, rorder=rorder, rank_of_row=rank_of_row))

    Lg = Lg_per_core.max(axis=0)                      # shared stripe lengths
    off = np.zeros(NGRP + 1, dtype=np.int64)
    off[1:] = np.cumsum(Lg)
    W = int(off[-1])
    # classes: runs of equal L
    classes = []
    g0 = 0
    for g in range(1, NGRP + 1):
        if g == NGRP or Lg[g] != Lg[g0]:
            classes.append((int(g0), int(g), int(off[g0]), int(Lg[g0])))
            g0 = g

    # per-core slot assignment
    for k in range(NCORES):
        d = per_core[k]
        rk = d["rank_of_row"][d["r"]]
        eorder = np.argsort(rk, kind="stable")        # edges sorted by rank
        rk_s = rk[eorder]
        uniq, counts = np.unique(rk_s, return_counts=True)
        starts = np.zeros(len(uniq), dtype=np.int64)
        starts[1:] = np.cumsum(counts)[:-1]
        pos = np.arange(len(rk_s)) - np.repeat(starts, counts)
        g = rk_s // P
        lane = rk_s % P
        wslot = off[g] + pos
        d.update(eorder=eorder, lane=lane, wslot=wslot)

    # ---- launch A: matvec on device ----
    nc_a = _build_launch_a()
    att4 = np.empty((P, 4), dtype=np.float32)
    att4[:, 0] = att[0:128]
    att4[:, 1] = att[256:384]
    att4[:, 2] = att[128:256]
    att4[:, 3] = att[384:512]
    in_maps_a = []
    for k in range(NCORES):
        xp = x[k * RPC + per_core[k]["rorder"], :]    # rank-ordered shard
        in_maps_a.append(
            dict(
                att4=att4,
                xh0=np.ascontiguousarray(xp[:, :128].T),
                xh1=np.ascontiguousarray(xp[:, 128:].T),
            )
        )
    res_a = run_bass_kernel_spmd(
        nc_a, in_maps_a, core_ids=list(range(NCORES)), trace=True
    )
    EXEC_NS["A"] = res_a.exec_time_ns

    # assemble s_dst in original node ids; keep s_src in rank order
    s_dst_all = np.empty(N_NODES, dtype=np.float32)
    ssrc_rank = []
    for k in range(NCORES):
        s = res_a.results[k]["s"]
        s_dst_all[k * RPC + per_core[k]["rorder"]] = s[1]
        sr = np.zeros(RPAD, dtype=np.float32)
        sr[:RPC] = s[0]
        ssrc_rank.append(sr)

    # ---- host reshard: expand s_dst values into the row-stripe layout ----
    nc_b = _build_launch_b(W, classes)
    in_maps_b = []
    for k in range(NCORES):
        d = per_core[k]
        b = np.full((P, W), NEG_BIG, dtype=np.float32)
        b[d["lane"], d["wslot"]] = s_dst_all[col[d["m"][d["eorder"]]]]
        in_maps_b.append(dict(bvals=b, ssrc=ssrc_rank[k]))
    res_b = run_bass_kernel_spmd(
        nc_b, in_maps_b, core_ids=list(range(NCORES)), trace=True
    )
    EXEC_NS["B"] = res_b.exec_time_ns

    # ---- host unshard: pick real slots back into original edge order ----
    out = np.empty(N_EDGES, dtype=np.float32)
    for k in range(NCORES):
        d = per_core[k]
        dev = res_b.results[k]["out"]
        out[d["m"][d["eorder"]]] = dev[d["lane"], d["wslot"]]
    return out[None, :]
